# revision 2
# baseline (speedup 1.0000x reference)
"""Depth-guided 3x3 convolution (nn_DepthConv) on 8 TRN2 NeuronCores.

Pixel-major two-phase design. Sharding: data-parallel over batch (B=8 -> 1
image per core), weights replicated, no collectives. Bias is added on host.

Per-core algorithm (image [C=64, H=128, W=128]):
  out[o,p] = sum_t W_t[o,c] * x[c, p+dt] * sim_t(p)

Layout: PACKED rows (stride W=128, no column padding). Channel-major split
halves [128, 68*128]: partitions 0-63 = channels for image rows -2..65
(half A), 64-127 = rows 62..129 (half B). Column wrap-around at row edges is
cancelled by zeros baked into the similarity maps; the extra lead pad row
keeps all shifted access patterns in-bounds.

Tap arrays: xbuf (center) + wp[k] (tap +d) + wm[k] (tap -d), k over
DELTAS = (0,1),(1,-1),(1,0),(1,1), dl = dh*128+dw.
  wp_d[q] = x[q]*m_d[q-d],  wm_d[q] = x[q]*m_d[q]  (m_d = 0 out of image)
Maps broadcast once into the wp slot (HBM scratch roundtrip, dual-row
partition-step-0 read fills both halves in one DMA); wm derives via the
shifted read wp_slot[q+dl] before wp is overwritten in place. All
multiplies on DVE (f16 2x mode), banded for pipelining.

Phase 1 matmuls are PIXEL-MAJOR: lhsT = tap array slice [64c, 128pix]
(stationary), rhs = wTp[c, t*64+o] (moving, free=64) -> psum [128pix, 64o],
9 taps accumulate; 8 rows of one half share a psum bank as column slots.
Phase 2: PE transposes [128pix, 64o] -> [64o, 128pix] (f16 psum), evacuate
to f16 stage, DMA out channel-major. Output dtype f16; host casts to f32
and adds bias.

The 16 row-tiles are emitted in 6 half-blocks (A/B alternating at block
granularity, paced to the multiply bands) because walrus cannot compile PE
streams whose accumulation groups alternate the contraction partition base
per-group. Transposes are software-pipelined one tile behind the matmuls so
their evac1 wait is pre-satisfied when the in-order PE queue reaches them.
"""

import sys

sys.path.insert(0, "/opt/trn_rl_repo")

import numpy as np

import concourse.bass as bass
import concourse.mybir as mybir
import concourse.bacc as bacc
import concourse.tile as tile
from concourse.bass_utils import run_bass_kernel_spmd
from concourse.masks import make_identity

F32 = mybir.dt.float32
F16 = mybir.dt.float16

C, O, H, W, KH, KW = 64, 64, 128, 128, 3, 3
ALPHA = 1.0
NR = 68               # flat rows per half (A: image rows -2..65, B: 62..129)
FR = NR * W           # 8704
DELTAS = [(0, 1), (1, -1), (1, 0), (1, 1)]
DLS = [dh * W + dw for (dh, dw) in DELTAS]       # 1, 127, 128, 129
BANDS = [(0, 12), (12, 20), (20, 28), (28, 36), (36, 44), (44, 52),
         (52, 60), (60, 68)]                     # tile-aligned flat-row bands
# (half, tile) emission order: tile m covers local rows 8m..8m+7 and needs
# flat rows <= 8m+10 (band m); blocks alternate halves at band pace
BLOCKS = [
    [(0, 0), (0, 1)], [(1, 0), (1, 1)],
    [(0, 2), (0, 3)], [(1, 2), (1, 3)],
    [(0, 4), (0, 5)], [(1, 4), (1, 5)],
    [(0, 6), (0, 7)], [(1, 6), (1, 7)],
]


def build_program():
    nc = bacc.Bacc("TRN2", target_bir_lowering=False, debug=False)

    x_t = nc.dram_tensor("x", [C, H, W], F32, kind="ExternalInput")
    d_t = nc.dram_tensor("depth", [1, H, W], F32, kind="ExternalInput")
    w_t = nc.dram_tensor("weight", [O, C, KH, KW], F32, kind="ExternalInput")
    out_t = nc.dram_tensor("out", [16, 2, 64, 512], F16, kind="ExternalOutput")
    scratch = nc.dram_tensor("mscratch", [8, FR], F16, kind="Internal")

    with tile.TileContext(nc) as tc:
        with (
            tc.tile_pool(name="big", bufs=1) as big,
            tc.tile_pool(name="small", bufs=1) as small,
            tc.tile_pool(name="mapp", bufs=4) as mapp,
            tc.tile_pool(name="psw", bufs=1, space="PSUM") as psw_pool,
            tc.tile_pool(name="ps1", bufs=4, space="PSUM") as ps1_pool,
            tc.tile_pool(name="ps2", bufs=3, space="PSUM") as ps2_pool,
            tc.tile_pool(name="stp", bufs=3) as stp_pool,
            tc.tile_pool(name="st2", bufs=3) as st2_pool,
        ):
            # ---------------- persistent SBUF tensors ----------------
            xbuf = big.tile([128, FR], F16, tag="xbuf")
            wp = [big.tile([128, FR], F16, tag=f"wp{k}", name=f"wp{k}")
                  for k in range(4)]
            wm = [big.tile([128, FR], F16, tag=f"wm{k}", name=f"wm{k}")
                  for k in range(4)]
            wTp = small.tile([128, 9 * O], F16, tag="wTp")
            w_raw = small.tile([64, C * KH * KW], F32, tag="wraw")
            dbuf = small.tile([128, W], F32, tag="dbuf")
            dsh = small.tile([128, W], F32, tag="dsh")
            idbig = small.tile([64, 64], F32, tag="idbig")
            identf = small.tile([128, 128], F16, tag="identf")
            idfull = small.tile([128, 128], F32, tag="idfull")
            mflat = small.tile([8, FR], F16, tag="mflat")

            xv = xbuf[:, :].rearrange("p (r w) -> p r w", r=NR)
            mfv = [
                mflat[i : i + 1, :].rearrange("p (r w) -> p r w", r=NR)
                for i in range(8)
            ]

            # ---------------- depth + maps first (critical path) ----------
            warm = small.tile([1, 8], F32, tag="warm")
            nc.vector.memset(warm[:, :], 0.0)
            nc.scalar.activation(
                warm[:, :], warm[:, :], mybir.ActivationFunctionType.Exp
            )
            nc.gpsimd.memset(dsh[:, :], 0.0)
            nc.sync.dma_start(out=dbuf[0:128, :], in_=d_t[0, :, :])
            nc.sync.dma_start(out=dsh[0:127, :], in_=d_t[0, 1:128, :])
            nc.scalar.dma_start(out=w_raw[:, :], in_=w_t[:, :, :, :])

            # sim maps in zero-bordered tiles: mt[:, 1+c] = m_d[:, c],
            # mt cols 0 / 129 zero; invalid edge col of the map also zeroed
            mtiles = []
            for k, (dh, dw) in enumerate(DELTAS):
                dsrc = dsh if dh == 1 else dbuf
                a = max(0, -dw)
                b = min(W, W - dw)
                diff = mapp.tile([128, W], F32, tag="diff")
                nc.vector.memset(diff[:, :], 0.0)
                nc.vector.tensor_sub(
                    diff[:, a:b], dsrc[:, a + dw : b + dw], dbuf[:, a:b]
                )
                absd = mapp.tile([128, W], F32, tag="absd")
                nc.scalar.activation(
                    absd[:, :], diff[:, :], mybir.ActivationFunctionType.Abs
                )
                mt = mapp.tile([128, W + 2], F16, tag=f"mt{k}")
                nc.scalar.activation(
                    mt[:, 1 : 1 + W], absd[:, :],
                    mybir.ActivationFunctionType.Exp, scale=-ALPHA,
                )
                # zero borders incl. the out-of-image map column
                if dw == 1:
                    nc.gpsimd.memset(mt[:, 0:1], 0.0)
                    nc.gpsimd.memset(mt[:, W : W + 2], 0.0)
                elif dw == -1:
                    nc.gpsimd.memset(mt[:, 0:2], 0.0)
                    nc.gpsimd.memset(mt[:, W + 1 : W + 2], 0.0)
                else:
                    nc.gpsimd.memset(mt[:, 0:1], 0.0)
                    nc.gpsimd.memset(mt[:, W + 1 : W + 2], 0.0)
                mtiles.append(mt)

            nc.vector.memset(mflat[:, 0 : 3 * W], 0.0)
            nc.vector.memset(mflat[:, 66 * W : FR], 0.0)

            # ---------------- flatten maps into mflat (shift baked) ----------
            # row 2k   (half A): [i, w] = m_d[i-2-dh, w-dw] = mtA[i-2-dh, 1+w-dw]
            # row 2k+1 (half B): [j, w] = m_d[62+j-dh, w-dw]
            for k, (dh, dw) in enumerate(DELTAS):
                mt = mtiles[k]
                nrA = 66 - dh
                nc.sync.dma_start(
                    out=mfv[2 * k][:, 2 + dh : 2 + dh + nrA, 0:W],
                    in_=mt[0:nrA, 1 - dw : 1 - dw + W],
                )
                nc.scalar.dma_start(
                    out=mfv[2 * k + 1][:, 0:66, 0:W],
                    in_=mt[62 - dh : 128 - dh, 1 - dw : 1 - dw + W],
                )
                qd = nc.sync if k % 2 == 0 else nc.scalar
                qd.dma_start(
                    out=scratch[2 * k : 2 * k + 2, :],
                    in_=mflat[2 * k : 2 * k + 2, :],
                )

            # x halo zeros (A: flat 0,1 <- image -2,-1; B: flat 66,67)
            nc.gpsimd.memset(xv[0:64, 0:2, :], 0.0)
            nc.gpsimd.memset(xv[64:128, 66:68, :], 0.0)

            # ---------------- identity + weights -> wTp ----------------
            make_identity(nc, idbig[:, :])
            make_identity(nc, idfull[:, :])
            nc.scalar.copy(out=identf[:, :], in_=idfull[:, :])
            for t in range(9):
                wps = psw_pool.tile([64, 64], F32, tag="wps", name="wps")
                nc.tensor.transpose(
                    wps[:, :], w_raw[:, t : C * 9 : 9], idbig[:, :]
                )
                nc.scalar.copy(out=wTp[0:64, t * O : (t + 1) * O], in_=wps[:, :])
            nc.scalar.dma_start(out=wTp[64:128, :], in_=wTp[0:64, :])


            # ---------------- per-band: bcast + x load + multiplies ----------
            # broadcast in 5 disjoint shifted windows (halo pre-included so
            # later windows never overlap earlier wm reads); multiplies stay
            # on the 8 fine bands; x loads in 4 just-in-time chunks
            WIN = {0: (0, 12), 12: (12, 28), 28: (28, 44), 44: (44, 60),
                   60: (60, 68)}
            XCH = {0: [(0, 2, 36), (1, 0, 34)], 20: [(1, 34, 66)],
                   28: [(0, 36, 68)]}
            for bi, (a, b) in enumerate(BANDS):
                lo, hi = a * W, b * W
                if a in WIN:
                    wa, wb = WIN[a]
                    blo = 0 if wa == 0 else wa * W + 130
                    bhi = min(wb * W + 130, FR)
                    for k in range(4):
                        bsrc = (
                            scratch[2 * k : 2 * k + 2, blo:bhi]
                            .partition_broadcast(64)
                            .transpose([1, 0, 2])
                        )
                        dma = nc.sync if k % 2 == 0 else nc.scalar
                        dma.dma_start(out=wp[k][:, blo:bhi], in_=bsrc)
                for (hf, r0x, r1x) in XCH.get(a, ()):
                    img0 = r0x - 2 if hf == 0 else r0x + 62
                    nc.gpsimd.dma_start(
                        out=xv[64 * hf : 64 * hf + 64, r0x:r1x, :],
                        in_=x_t[:, img0 : img0 + (r1x - r0x), :],
                    )
                # wm then wp (in place) multiplies; one wp op per band goes
                # to the otherwise-idle Pool engine
                for k in range(4):
                    dl = DLS[k]
                    mhi = min(hi, FR - dl)
                    nc.vector.tensor_tensor(
                        out=wm[k][:, lo:mhi],
                        in0=xbuf[:, lo:mhi],
                        in1=wp[k][:, lo + dl : mhi + dl],
                        op=mybir.AluOpType.mult,
                    )
                for k in range(4):
                    eng = nc.gpsimd if k == 3 else nc.vector
                    eng.tensor_mul(
                        wp[k][:, lo:hi], xbuf[:, lo:hi], wp[k][:, lo:hi]
                    )

            # ---------------- phase 1 + 2 main loop ----------------
            # taps: (array, dh_read, dw_read, weight slot kh*3+kw)
            taps = [(xbuf, 0, 0, 4)]
            for k, (dh, dw) in enumerate(DELTAS):
                taps.append((wp[k], dh, dw, (dh + 1) * 3 + (dw + 1)))
                taps.append((wm[k], -dh, -dw, (1 - dh) * 3 + (1 - dw)))

            def emit_phase2(stp, half, m, di):
                ps2 = ps2_pool.tile([64, 1024], F16, tag="ps2", name="ps2")
                for j in range(8):
                    nc.tensor.transpose(
                        ps2[:, 128 * j : 128 * j + 128],
                        stp[:, 64 * j : 64 * j + 64],
                        identf[:, :],
                    )
                st2 = st2_pool.tile([64, 1024], F16, tag="st2", name="st2")
                nc.scalar.copy(out=st2[:, :], in_=ps2[:, :])
                dma = nc.sync if di % 4 == 0 else (
                    nc.scalar if di % 4 == 2 else nc.gpsimd)
                dma.dma_start(
                    out=out_t[8 * half + m, :, :, :].transpose([1, 0, 2]),
                    in_=st2[:, :],
                )

            pending = None
            di = 0
            for block in BLOCKS:
                for (half, m) in block:
                    p0 = 64 * half
                    ps1 = ps1_pool.tile([128, 512], F32, tag="ps1", name="ps1")
                    for j in range(8):
                        r = 8 * m + j
                        for t, (arr, dh, dw, slot) in enumerate(taps):
                            off = (r + 2 + dh) * W + dw
                            nc.tensor.matmul(
                                ps1[:, 64 * j : 64 * j + 64],
                                arr[p0 : p0 + 64, off : off + 128],
                                wTp[p0 : p0 + 64, slot * O : (slot + 1) * O],
                                start=(t == 0), stop=(t == 8),
                            )
                    if pending is not None:
                        emit_phase2(*pending, di)
                        di += 2
                    stp = stp_pool.tile([128, 512], F16, tag="stp", name="stp")
                    nc.scalar.copy(out=stp[:, :], in_=ps1[:, :])
                    pending = (stp, half, m)
            emit_phase2(*pending, di)

    nc.compile()
    return nc


_NC_CACHE = None
_WARMED = False


def _get_nc():
    global _NC_CACHE
    if _NC_CACHE is None:
        _NC_CACHE = build_program()
    return _NC_CACHE


def kernel(x, depth, weight, bias):
    x = np.asarray(x, dtype=np.float32)
    depth = np.asarray(depth, dtype=np.float32)
    weight = np.asarray(weight, dtype=np.float32)
    bias = np.asarray(bias, dtype=np.float32)
    B = x.shape[0]
    assert B == 8
    nc = _get_nc()
    in_maps = [
        {"x": x[b], "depth": depth[b], "weight": weight}
        for b in range(B)
    ]
    global _WARMED
    if not _WARMED:
        run_bass_kernel_spmd(nc, in_maps, core_ids=list(range(B)))
        _WARMED = True
    res = run_bass_kernel_spmd(nc, in_maps, core_ids=list(range(B)))

    def reasm(v):
        # v: [16, 2, 64, 512]; tile t = 8h+m holds st2 [64 o, 1024 = (8r, 128c)]
        # stored as [2, 64, 512]; row index r = 8m + j
        v = v.reshape(2, 8, 2, 64, 4, 128).transpose(3, 0, 1, 2, 4, 5)
        return v.reshape(64, 128, 128).astype(np.float32)

    out = np.stack([reasm(res.results[b]["out"]) for b in range(B)], axis=0)
    return out + bias[None, :, None, None]


if __name__ == "__main__":
    rng = np.random.default_rng(0)
    x = rng.standard_normal((8, C, H, W), dtype=np.float32)
    d = rng.random((8, 1, H, W), dtype=np.float32)
    w = rng.standard_normal((O, C, KH, KW), dtype=np.float32) * 0.04
    b = rng.standard_normal((O,), dtype=np.float32) * 0.04
    out = kernel(x=x, depth=d, weight=w, bias=b)
    print(out.shape, out.dtype)
